# revision 48
# baseline (speedup 1.0000x reference)
"""Trainium2 Bass kernel for MultiHeadedAttention with learned memory slots +
attention-weight logit modulation + residual LayerNorm.

Sharding: data-parallel over batch — 16 batches across 8 cores (2 per core).
Each core runs an identical single-core Bass program (SPMD, no collectives).

Device-side strategy (per core, per batch):
  - Host pre-transposes activations so every matmul contraction dim lands on
    SBUF partitions with fast contiguous DMAs (no on-chip transposes).
  - Attention runs in "S^T" orientation: S^T[k, q] tiles with k on partitions,
    so P^T = exp(w^T * S^T) feeds P@V directly (V stationary, P^T moving) and
    O^T[hd, q] feeds the output projection directly as the stationary operand.
  - Softmax denominators come free from an extra ones-column in the PV
    stationary operand; normalization is applied to O^T afterwards (reciprocal
    via the DVE bit-trick op, partition-broadcast via a DRAM bounce).
  - LayerNorm rstd = exp(-0.5*ln(var+eps)) and the activation-table pass is
    pinned to the combined natural_log_exp_and_others set: one table load.
  - Batches are software-pipelined: batch b+1's projections and batch b's
    LayerNorm tail are interleaved into batch b's attention stream so PE fills
    the gaps left by the DVE/ACT-bound softmax pipeline.
"""

import os
import sys

import numpy as np

for _p in ("/root/.axon_site/_ro/trn_rl_repo", "/opt/trn_rl_repo"):
    if os.path.isdir(_p) and _p not in sys.path:
        sys.path.append(_p)

import concourse.bass as bass
import concourse.bacc as bacc
import concourse.mybir as mybir
import concourse.tile as tile
from concourse.bass_utils import run_bass_kernel_spmd

F32 = mybir.dt.float32
BF16 = mybir.dt.bfloat16
AF = mybir.ActivationFunctionType
ALU = mybir.AluOpType

N_CORES = 8
B_TOT, NQ, D = 16, 1024, 512
NK, H, DK, MSLOT = 1024, 8, 64, 40
BPC = B_TOT // N_CORES  # batches per core
NKM = NK + MSLOT
LN_EPS = 1e-3

_CACHE = {}


def _build_module(nq=NQ, nk=NK, repeat=1, trivial=False):
    """trivial=True compiles the fast path for bq=bk=0, gamma=1, beta=0
    (bv/bo are folded host-side in all cases): the LayerNorm gamma/beta
    GpSimd ops disappear and the final scale lands on GpSimd to shorten
    the DVE tail."""
    NQL, NKL = nq, nk
    NKML = nk + MSLOT
    QBLK = min(512, NQL)  # q columns per matmul/psum block
    NQB = NQL // QBLK  # q blocks
    NQT = NQL // 128  # q 128-tiles
    KTF = NKL // 128  # full k tiles (w-modulated region)
    nc = bacc.Bacc("TRN2", target_bir_lowering=False, debug=False)

    qT = nc.dram_tensor("qT", [BPC, D, NQL], BF16, kind="ExternalInput")
    kTin = nc.dram_tensor("kTin", [BPC, D, NKL], BF16, kind="ExternalInput")
    vTin = nc.dram_tensor("vTin", [BPC, D, NKL], BF16, kind="ExternalInput")
    wT = nc.dram_tensor("wT", [BPC, NKL, NQL], BF16, kind="ExternalInput")
    qres = nc.dram_tensor("qres", [BPC, NQL, D], BF16, kind="ExternalInput")
    wq = nc.dram_tensor("wq", [D, D], BF16, kind="ExternalInput")
    wk = nc.dram_tensor("wk", [D, D], BF16, kind="ExternalInput")
    wv = nc.dram_tensor("wv", [D, D], BF16, kind="ExternalInput")
    wo = nc.dram_tensor("wo", [D, D], BF16, kind="ExternalInput")
    bqv = nc.dram_tensor("bqv", [D], F32, kind="ExternalInput")
    bkv = nc.dram_tensor("bkv", [D], F32, kind="ExternalInput")
    bvv = nc.dram_tensor("bvv", [D], F32, kind="ExternalInput")
    memkT = nc.dram_tensor("memkT", [D, MSLOT], BF16, kind="ExternalInput")
    memv = nc.dram_tensor("memv", [MSLOT, D], BF16, kind="ExternalInput")
    gam = nc.dram_tensor("gam", [D], F32, kind="ExternalInput")
    bet = nc.dram_tensor("bet", [D], F32, kind="ExternalInput")
    out = nc.dram_tensor("out", [BPC, NQL, D], BF16, kind="ExternalOutput")

    def bcast_row(dram_vec, parts=128):
        ap = dram_vec[:]
        return bass.AP(tensor=ap.tensor, offset=ap.offset, ap=[[0, parts], ap.ap[0]])

    with tile.TileContext(nc) as tc:
        import contextlib

        ctx = contextlib.ExitStack()
        with ctx:
            singles = ctx.enter_context(tc.tile_pool(name="singles", bufs=1))
            xin = ctx.enter_context(tc.tile_pool(name="xin", bufs=3))
            p_qt = ctx.enter_context(tc.tile_pool(name="p_qt", bufs=2))
            p_kt = ctx.enter_context(tc.tile_pool(name="p_kt", bufs=2))
            p_v = ctx.enter_context(tc.tile_pool(name="p_v", bufs=2))
            p_wt = ctx.enter_context(tc.tile_pool(name="p_wt", bufs=1))
            p_ot = ctx.enter_context(tc.tile_pool(name="p_ot", bufs=2))
            p_p = ctx.enter_context(tc.tile_pool(name="p_p", bufs=2))
            p_den = ctx.enter_context(tc.tile_pool(name="p_den", bufs=2))
            p_r = ctx.enter_context(tc.tile_pool(name="p_r", bufs=1))
            p_small = ctx.enter_context(tc.tile_pool(name="p_small", bufs=3))
            ps_s = ctx.enter_context(tc.tile_pool(name="ps_s", bufs=2, space="PSUM"))
            ps_pv = ctx.enter_context(tc.tile_pool(name="ps_pv", bufs=2, space="PSUM"))
            ps_pr = ctx.enter_context(tc.tile_pool(name="ps_pr", bufs=2, space="PSUM"))
            p_dram = ctx.enter_context(
                tc.tile_pool(name="p_dram", bufs=2, space="DRAM")
            )

            # --- persistent weights/constants ---
            wq_sb = singles.tile([128, 4, D], BF16, tag="wq")
            wk_sb = singles.tile([128, 4, D], BF16, tag="wk")
            wv_sb = singles.tile([128, 4, D], BF16, tag="wv")
            wo_sb = singles.tile([128, 4, D], BF16, tag="wo")
            nc.sync.dma_start(out=wq_sb, in_=wq[:, :].rearrange("(c p) d -> p c d", p=128))
            nc.sync.dma_start(out=wk_sb, in_=wk[:, :].rearrange("(c p) d -> p c d", p=128))
            nc.sync.dma_start(out=wv_sb, in_=wv[:, :].rearrange("(c p) d -> p c d", p=128))
            nc.sync.dma_start(out=wo_sb, in_=wo[:, :].rearrange("(c p) d -> p c d", p=128))
            bq_sb = singles.tile([128, 4], F32, tag="bq")
            bk_sb = singles.tile([128, 4], F32, tag="bk")
            nc.sync.dma_start(out=bq_sb, in_=bqv[:].rearrange("(t p) -> p t", p=128))
            nc.sync.dma_start(out=bk_sb, in_=bkv[:].rearrange("(t p) -> p t", p=128))
            bv_bc = singles.tile([128, D], F32, tag="bv")
            nc.sync.dma_start(out=bv_bc, in_=bcast_row(bvv))
            gam_bc = singles.tile([128, D], F32, tag="gam")
            bet_bc = singles.tile([128, D], F32, tag="bet")
            nc.sync.dma_start(out=gam_bc, in_=bcast_row(gam))
            nc.sync.dma_start(out=bet_bc, in_=bcast_row(bet))
            eps_t = singles.tile([128, 1], F32, tag="eps")
            nc.vector.memset(eps_t, LN_EPS)

            def load_batch(b):
                t = {}
                t["qT_in"] = xin.tile([128, 4, NQL], BF16, tag="xin", name="qT_in")
                t["kT_in"] = xin.tile([128, 4, NKL], BF16, tag="xin", name="kT_in")
                t["vT_in"] = xin.tile([128, 4, NKL], BF16, tag="xin", name="vT_in")
                nc.sync.dma_start(
                    out=t["qT_in"], in_=qT[b].rearrange("(c p) q -> p c q", p=128)
                )
                nc.sync.dma_start(
                    out=t["kT_in"], in_=kTin[b].rearrange("(c p) q -> p c q", p=128)
                )
                t["wt"] = p_wt.tile([128, KTF, NQL], BF16, tag="wt", name="wt_sb")
                wsrc = wT[b].rearrange("(t p) q -> p t q", p=128)
                nc.sync.dma_start(out=t["wt"][:, 0:2, :], in_=wsrc[:, 0:2, :])
                nc.sync.dma_start(
                    out=t["vT_in"], in_=vTin[b].rearrange("(c p) q -> p c q", p=128)
                )
                nc.sync.dma_start(out=t["wt"][:, 2:KTF, :], in_=wsrc[:, 2:KTF, :])
                t["qt"] = p_qt.tile([128, 4, NQL], BF16, tag="qt", name="qt_slab")
                t["kt"] = p_kt.tile([128, 4, NKML], BF16, tag="kt", name="kt_slab")
                t["v"] = p_v.tile([128, KTF + 1, H, DK + 1], BF16, tag="v", name="v_slab")
                t["ot"] = p_ot.tile([128, 4, NQL], BF16, tag="ot", name="ot_slab")
                nc.sync.dma_start(
                    out=t["kt"][:, :, NKL:NKML],
                    in_=memkT[:, :].rearrange("(c p) m -> p c m", p=128),
                )
                nc.sync.dma_start(
                    out=t["v"][0:MSLOT, KTF, :, 0:DK],
                    in_=memv[:, :].rearrange("k (h d) -> k h d", h=H),
                )
                nc.vector.memset(t["v"][:, :, :, DK], 1.0)
                return t

            def proj_gen(b, t):
                def qk_chunks(dt_i):
                    for qb in range(NQB):
                        ps = ps_pr.tile([128, QBLK], F32, tag="pr")
                        for ct in range(4):
                            nc.tensor.matmul(
                                ps,
                                lhsT=wq_sb[:, ct, dt_i * 128 : (dt_i + 1) * 128],
                                rhs=t["qT_in"][:, ct, qb * QBLK : (qb + 1) * QBLK],
                                start=(ct == 0),
                                stop=(ct == 3),
                            )
                        nc.scalar.activation(
                            out=t["qt"][:, dt_i, qb * QBLK : (qb + 1) * QBLK],
                            in_=ps,
                            func=AF.Identity,
                            bias=bq_sb[:, dt_i : dt_i + 1],
                            scale=1.0,
                        )
                        yield
                    for qb in range(max(1, NKL // QBLK)):
                        ps = ps_pr.tile([128, QBLK], F32, tag="pr")
                        for ct in range(4):
                            nc.tensor.matmul(
                                ps,
                                lhsT=wk_sb[:, ct, dt_i * 128 : (dt_i + 1) * 128],
                                rhs=t["kT_in"][:, ct, qb * QBLK : (qb + 1) * QBLK],
                                start=(ct == 0),
                                stop=(ct == 3),
                            )
                        nc.scalar.activation(
                            out=t["kt"][:, dt_i, qb * QBLK : (qb + 1) * QBLK],
                            in_=ps,
                            func=AF.Identity,
                            bias=bk_sb[:, dt_i : dt_i + 1],
                            scale=1.0,
                        )
                        yield

                def v_chunks():
                    for kt_i in range(KTF):
                        ps = ps_pr.tile([128, D], F32, tag="pr")
                        for ct in range(4):
                            nc.tensor.matmul(
                                ps,
                                lhsT=t["vT_in"][:, ct, kt_i * 128 : (kt_i + 1) * 128],
                                rhs=wv_sb[:, ct, :],
                                start=(ct == 0),
                                stop=(ct == 3),
                            )
                        nc.scalar.copy(
                            out=t["v"][:, kt_i, :, 0:DK],
                            in_=ps.rearrange("p (h d) -> p h d", h=H),
                        )
                        yield

                yield from qk_chunks(0)
                yield from v_chunks()
                for dt_i in range(1, 4):
                    yield from qk_chunks(dt_i)

            def attn_gen(b, t):
                for qb in range(NQB):
                    qsl = slice(qb * QBLK, (qb + 1) * QBLK)

                    den = p_den.tile([128, 2, QBLK], F32, tag="den")
                    nc.vector.memset(den, 1.0)

                    pv_jobs = []
                    scratch = p_dram.tile([H, QBLK], F32, tag="scr", name="scr")
                    r_slab = p_r.tile([128, 4, QBLK], F32, tag="r", name="r_slab")
                    pv_done = [0]

                    def finish_slot(slot):
                        # heads 4*slot..4*slot+3 have their denominators in
                        # den[:, slot, :]; reciprocal + DRAM-bounce broadcast
                        nc.vector.reciprocal_approx_fast(
                            den[:, slot, :], den[:, slot, :]
                        )
                        for h in range(4 * slot, 4 * slot + 4):
                            nc.sync.dma_start(
                                out=scratch[h, :],
                                in_=den[32 * (h % 4) : 32 * (h % 4) + 1, h // 4, :],
                            )
                        for h in range(4 * slot, 4 * slot + 4):
                            nc.sync.dma_start(
                                out=r_slab[
                                    64 * (h % 2) : 64 * (h % 2) + 64, h // 2, :
                                ],
                                in_=scratch[h : h + 1, :].to_broadcast((64, QBLK)),
                            )

                    def do_pv(pair, ppair):
                        for half in range(2):
                            h = 2 * pair + half
                            pspv = ps_pv.tile([DK + 1, QBLK], F32, tag="pv")
                            for kt_i in range(KTF + 1):
                                ksz = 128 if kt_i < KTF else MSLOT
                                nc.tensor.matmul(
                                    pspv[0 : DK + 1, :],
                                    lhsT=t["v"][0:ksz, kt_i, h, 0 : DK + 1],
                                    rhs=ppair[0:ksz, half, kt_i, :],
                                    start=(kt_i == 0),
                                    stop=(kt_i == KTF),
                                )
                            nc.scalar.copy(
                                out=den[32 * (h % 4) : 32 * (h % 4) + 1, h // 4, :],
                                in_=pspv[DK : DK + 1, :],
                            )
                            nc.scalar.copy(
                                out=t["ot"][64 * half : 64 * half + 64, pair, qsl],
                                in_=pspv[0:DK, :],
                            )
                        pv_done[0] += 1
                        if pv_done[0] == 2:
                            finish_slot(0)
                        elif pv_done[0] == 4:
                            finish_slot(1)

                    for pair in range(4):
                        ppair = p_p.tile([128, 2, KTF + 1, QBLK], BF16, tag="pp")
                        for ktg in range(KTF // 4):
                            for kt_i in range(4 * ktg, 4 * ktg + 4):
                                ps = ps_s.tile([128, 2, QBLK], F32, tag="s")
                                for half in range(2):
                                    nc.tensor.matmul(
                                        ps[:, half, :],
                                        lhsT=t["kt"][
                                            64 * half : 64 * half + 64,
                                            pair,
                                            kt_i * 128 : (kt_i + 1) * 128,
                                        ],
                                        rhs=t["qt"][
                                            64 * half : 64 * half + 64, pair, qsl
                                        ],
                                        start=True,
                                        stop=True,
                                    )
                                w_b = (
                                    t["wt"][:, kt_i, qsl]
                                    .unsqueeze(1)
                                    .to_broadcast((128, 2, QBLK))
                                )
                                nc.vector.tensor_tensor(
                                    out=ppair[:, :, kt_i, :],
                                    in0=ps,
                                    in1=w_b,
                                    op=ALU.mult,
                                )
                            nc.scalar.activation(
                                out=ppair[:, :, 4 * ktg : 4 * ktg + 4, :],
                                in_=ppair[:, :, 4 * ktg : 4 * ktg + 4, :],
                                func=AF.Exp,
                            )
                        ps = ps_s.tile([128, 2, QBLK], F32, tag="s")
                        for half in range(2):
                            nc.tensor.matmul(
                                ps[0:MSLOT, half, :],
                                lhsT=t["kt"][64 * half : 64 * half + 64, pair, NKL:NKML],
                                rhs=t["qt"][64 * half : 64 * half + 64, pair, qsl],
                                start=True,
                                stop=True,
                            )
                        nc.scalar.activation(
                            out=ppair[0:MSLOT, :, KTF, :],
                            in_=ps[0:MSLOT, :, :],
                            func=AF.Exp,
                        )
                        pv_jobs.append((pair, ppair))
                        if len(pv_jobs) >= 2:
                            do_pv(*pv_jobs.pop(0))
                        yield ("pair", qb)
                    while pv_jobs:
                        do_pv(*pv_jobs.pop(0))

                    nc.vector.tensor_tensor(
                        out=t["ot"][:, :, qsl],
                        in0=t["ot"][:, :, qsl],
                        in1=r_slab,
                        op=ALU.mult,
                    )
                    yield ("tail", qb)

            def out_gen(b, t):
                for qt_i in range(NQT):
                    psy = ps_pr.tile([128, D], F32, tag="pr")
                    for p4 in range(4):
                        nc.tensor.matmul(
                            psy,
                            lhsT=t["ot"][:, p4, qt_i * 128 : (qt_i + 1) * 128],
                            rhs=wo_sb[:, p4, :],
                            start=(p4 == 0),
                            stop=(p4 == 3),
                        )
                    qr = p_small.tile([128, D], BF16, tag="qr")
                    nc.sync.dma_start(
                        out=qr, in_=qres[b, qt_i * 128 : (qt_i + 1) * 128, :]
                    )
                    x_t = p_small.tile([128, D], F32, tag="x")
                    nc.vector.tensor_tensor(out=x_t, in0=psy, in1=qr, op=ALU.add)
                    stats = p_small.tile([128, 6], F32, tag="st")
                    nc.vector.bn_stats(stats, x_t)
                    mv = p_small.tile([128, 2], F32, tag="mv")
                    nc.vector.bn_aggr(mv, stats)
                    lnv = p_small.tile([128, 1], F32, tag="lnv")
                    nc.scalar.activation(
                        lnv, mv[:, 1:2], AF.Ln, bias=eps_t[:, 0:1], scale=1.0
                    )
                    rstd = p_small.tile([128, 1], F32, tag="rstd")
                    nc.scalar.activation(rstd, lnv, AF.Exp, scale=-0.5)
                    o_t = p_small.tile([128, D], BF16, tag="o")
                    if trivial:
                        # gamma==1, beta==0: (x-mu)*rstd is the output.
                        nc.vector.scalar_tensor_tensor(
                            out=o_t,
                            in0=x_t,
                            scalar=mv[:, 0:1],
                            in1=rstd[:, 0:1].to_broadcast((128, D)),
                            op0=ALU.subtract,
                            op1=ALU.mult,
                        )
                    else:
                        t_t = p_small.tile([128, D], F32, tag="t")
                        nc.vector.scalar_tensor_tensor(
                            out=t_t,
                            in0=x_t,
                            scalar=mv[:, 0:1],
                            in1=rstd[:, 0:1].to_broadcast((128, D)),
                            op0=ALU.subtract,
                            op1=ALU.mult,
                        )
                        og_t = p_small.tile([128, D], F32, tag="og")
                        nc.gpsimd.tensor_tensor(
                            out=og_t, in0=t_t, in1=gam_bc, op=ALU.mult
                        )
                        nc.gpsimd.tensor_tensor(out=o_t, in0=og_t, in1=bet_bc, op=ALU.add)
                    nc.sync.dma_start(
                        out=out[b, qt_i * 128 : (qt_i + 1) * 128, :], in_=o_t
                    )
                    yield

            def pump(gen, n):
                if gen is None:
                    return
                for _ in range(n):
                    try:
                        next(gen)
                    except StopIteration:
                        return

            def flush(gen):
                if gen is None:
                    return
                for _ in gen:
                    pass

            # ---------------- software-pipelined batch driver ----------------
            bseq = [bb for _ in range(repeat) for bb in range(BPC)]
            cur = load_batch(bseq[0])
            pcur = proj_gen(bseq[0], cur)
            # emit only the dt0 Q/K chunks (enough for attention pair 0); the
            # rest is spread behind the first q-block's pair markers: V + dt1
            # must land before PV(0)/QK(1), dt2 before QK(2), dt3 before QK(3)
            nqk = NQB + max(1, NKL // QBLK)
            pump(pcur, nqk)
            b0_sched = []
            prev_out = None
            for i, b in enumerate(bseq):
                t = cur
                nxt = pnext = None
                if i + 1 < len(bseq):
                    nxt = load_batch(bseq[i + 1])
                    pnext = proj_gen(bseq[i + 1], nxt)
                og = out_gen(b, t)
                og_allowed = 0
                og_pumped = 0
                last = i + 1 >= len(bseq)
                sched = list(b0_sched) if i == 0 else []
                for kind, qb in attn_gen(b, t):
                    if sched:
                        pump(pcur, sched.pop(0))
                    elif i == 0:
                        flush(pcur)
                    pump(pnext, 3)
                    pump(prev_out, 3)
                    if kind == "tail":
                        og_allowed += NQT // NQB
                    take = min(2 if last else 1, og_allowed - og_pumped)
                    if take > 0:
                        pump(og, take)
                        og_pumped += take
                flush(prev_out)
                flush(pcur)
                prev_out = og
                cur = nxt
                pcur = pnext
            flush(prev_out)

    # Pin the activation-table pass to the single combined set so Exp/Ln/
    # Identity/Copy never trigger table reloads.
    import concourse.hw_specs as hw_specs

    orig_tables = hw_specs.get_activation_tables(nc.m.arch)
    combined = "natural_log_exp_and_others"
    patched = {
        name: (funcs if name == combined else set())
        for name, funcs in orig_tables.items()
    }
    orig_fn = hw_specs.get_activation_tables
    import concourse.bacc as bacc_mod

    try:
        hw_specs.get_activation_tables = lambda arch: patched
        if hasattr(bacc_mod, "get_activation_tables"):
            bacc_mod.get_activation_tables = hw_specs.get_activation_tables
        nc.compile()
    finally:
        hw_specs.get_activation_tables = orig_fn
        if hasattr(bacc_mod, "get_activation_tables"):
            bacc_mod.get_activation_tables = orig_fn
    return nc


def get_module(nq=NQ, nk=NK, repeat=1, trivial=True):
    key = ("nc", nq, nk, repeat, trivial)
    if key not in _CACHE:
        _CACHE[key] = _build_module(nq, nk, repeat, trivial)
    return _CACHE[key]


def is_trivial(inputs) -> bool:
    return bool(
        not np.any(np.asarray(inputs["bq"]))
        and not np.any(np.asarray(inputs["bk"]))
        and not np.any(np.asarray(inputs["beta"]))
        and np.all(np.asarray(inputs["gamma"]) == 1.0)
    )


def make_in_maps(inputs):
    import ml_dtypes
    from concurrent.futures import ThreadPoolExecutor

    bf = ml_dtypes.bfloat16
    f32 = np.float32

    queries = np.asarray(inputs["queries"], f32)
    keys = np.asarray(inputs["keys"], f32)
    values = np.asarray(inputs["values"], f32)
    attw = np.asarray(inputs["attention_weights"], f32)
    Wq = np.asarray(inputs["Wq"], f32)
    Wk = np.asarray(inputs["Wk"], f32)
    Wv = np.asarray(inputs["Wv"], f32)
    Wo = np.asarray(inputs["Wo"], f32)
    bq = np.asarray(inputs["bq"], f32)
    bk = np.asarray(inputs["bk"], f32)
    bv = np.asarray(inputs["bv"], f32)
    bo = np.asarray(inputs["bo"], f32)
    memK = np.asarray(inputs["memK"], f32)
    memV = np.asarray(inputs["memV"], f32)
    gamma = np.asarray(inputs["gamma"], f32)
    beta = np.asarray(inputs["beta"], f32)

    scale = 1.0 / np.sqrt(DK).astype(f32)  # 0.125
    # exact folds: attn_out = sum_k phat_k*(v'_k) + bv with v' = raw Wv matmul
    # for data keys and (sqrt(M)*memV - bv) for memory slots; bv@Wo + bo then
    # join the residual.  The device never adds bv/bo.
    res_bias = (bo + bv @ Wo).astype(f32)
    wq_s = (Wq * scale).astype(bf)
    bq_s = (bq * scale).astype(f32)
    memkTh = np.ascontiguousarray((np.sqrt(DK).astype(f32) * memK[0]).T).astype(bf)
    memvh = (np.sqrt(MSLOT).astype(f32) * memV[0] - bv[None, :]).astype(bf)

    shared = {
        "wq": wq_s,
        "wk": Wk.astype(bf),
        "wv": Wv.astype(bf),
        "wo": Wo.astype(bf),
        "bqv": bq_s,
        "bkv": bk.astype(f32),
        "bvv": np.zeros_like(bv),
        "memkT": memkTh,
        "memv": memvh,
        "gam": gamma.astype(f32),
        "bet": beta.astype(f32),
    }

    def prep_core(c):
        sl = slice(c * BPC, (c + 1) * BPC)
        m = dict(shared)
        m["qT"] = np.ascontiguousarray(queries[sl].transpose(0, 2, 1)).astype(bf)
        m["kTin"] = np.ascontiguousarray(keys[sl].transpose(0, 2, 1)).astype(bf)
        m["vTin"] = np.ascontiguousarray(values[sl].transpose(0, 2, 1)).astype(bf)
        m["wT"] = np.ascontiguousarray(attw[sl, 0].transpose(0, 2, 1)).astype(bf)
        m["qres"] = (queries[sl] + res_bias[None, None, :]).astype(bf)
        return m

    with ThreadPoolExecutor(max_workers=8) as ex:
        in_maps = list(ex.map(prep_core, range(N_CORES)))
    return in_maps


_DISPATCH = {}


def _get_dispatcher(nc):
    """Build (once) a cached jitted SPMD dispatcher for the module, so repeat
    kernel() calls skip jax retracing/lowering."""
    key = id(nc)
    if key in _DISPATCH:
        return _DISPATCH[key]
    import jax
    import concourse.mybir as mybir_mod
    from concourse import bass2jax
    from jax.sharding import Mesh, PartitionSpec
    from jax.experimental.shard_map import shard_map

    bass2jax.install_neuronx_cc_hook()

    in_names, out_names, out_avals, zero_outs = [], [], [], []
    partition_name = nc.partition_id_tensor.name if nc.partition_id_tensor else None
    for alloc in nc.m.functions[0].allocations:
        if not isinstance(alloc, mybir_mod.MemoryLocationSet):
            continue
        name = alloc.memorylocations[0].name
        if alloc.kind == "ExternalInput":
            if name != partition_name:
                in_names.append(name)
        elif alloc.kind == "ExternalOutput":
            out_names.append(name)
            shape = tuple(alloc.tensor_shape)
            dtype = mybir_mod.dt.np(alloc.dtype)
            out_avals.append(jax.core.ShapedArray(shape, dtype))
            zero_outs.append(np.zeros(shape, dtype))
    n_params = len(in_names)
    all_names = list(in_names) + list(out_names)
    if partition_name is not None:
        all_names.append(partition_name)
    donate = tuple(range(n_params, n_params + len(out_names)))

    def _body(*args):
        operands = list(args)
        if partition_name is not None:
            operands.append(bass2jax.partition_id_tensor())
        outs = bass2jax._bass_exec_p.bind(
            *operands,
            out_avals=tuple(out_avals),
            in_names=tuple(all_names),
            out_names=tuple(out_names),
            lowering_input_output_aliases=(),
            sim_require_finite=True,
            sim_require_nnan=True,
            nc=nc,
        )
        return tuple(outs)

    devices = jax.devices()[:N_CORES]
    mesh = Mesh(np.asarray(devices), ("core",))
    in_specs = (PartitionSpec("core"),) * (n_params + len(out_names))
    out_specs = (PartitionSpec("core"),) * len(out_names)
    sharded = jax.jit(
        shard_map(
            _body, mesh=mesh, in_specs=in_specs, out_specs=out_specs, check_rep=False
        ),
        donate_argnums=donate,
        keep_unused=True,
    )
    # Output buffers are donated operands; make the zeros ON DEVICE so each
    # call doesn't push tens of MB of zeros through the transport.
    import jax.numpy as jnp
    from jax.sharding import NamedSharding

    zero_shapes = [
        ((N_CORES * z.shape[0], *z.shape[1:]), z.dtype) for z in zero_outs
    ]
    zeros_maker = jax.jit(
        lambda: tuple(jnp.zeros(s, d) for s, d in zero_shapes),
        out_shardings=tuple(
            NamedSharding(mesh, PartitionSpec("core")) for _ in zero_shapes
        ),
    )
    disp = (sharded, in_names, out_names, out_avals, zeros_maker)
    _DISPATCH[key] = disp
    return disp


def run_cached(nc, in_maps):
    sharded, in_names, out_names, out_avals, zeros_maker = _get_dispatcher(nc)
    concat_in = [
        np.concatenate([np.asarray(in_maps[c][n]) for c in range(N_CORES)], axis=0)
        for n in in_names
    ]
    out_arrs = sharded(*concat_in, *zeros_maker())
    return [
        {
            name: np.asarray(out_arrs[i]).reshape(N_CORES, *out_avals[i].shape)[c]
            for i, name in enumerate(out_names)
        }
        for c in range(N_CORES)
    ]


def kernel(**inputs) -> np.ndarray:
    nq = np.asarray(inputs["queries"]).shape[1]
    nk = np.asarray(inputs["keys"]).shape[1]
    nc = get_module(nq, nk, trivial=is_trivial(inputs))
    in_maps = make_in_maps(inputs)
    results = run_cached(nc, in_maps)
    out = np.concatenate([results[c]["out"] for c in range(N_CORES)], axis=0)
    return out.astype(np.float32)

